# revision 3
# baseline (speedup 1.0000x reference)
"""Bass/Trainium2 kernel for LinearRowShared4Bit.

y[b,s,o] = sum_i x[b,s,i] * W[o,i] + bias[o]
W[o,i]   = (2*q[o,i]/15 - 1) * norm[o//32, i//32]   (q = 4-bit nibbles)

Sharding: out_features (11008) split 1376-per-core across 8 cores; x replicated.

Host does layout only (transpose / nibble->byte unpack / norm expansion);
all dequant arithmetic ((q-7.5)*(2*norm/15)) and the matmul run on device.

Device per core:
  - dequantize W into a resident SBUF tile [128, 32, 1376] fp16 (11.3 MB)
  - for each 128-token tile: accumulate 3 o-chunks x 32 k-tile matmuls in PSUM
    (lhsT = xT k-tile [128,128], rhs = W k-tile chunk [128,<=512]),
    add bias on DVE, DMA out fp32 rows.
"""

import numpy as np

IN_F = 4096
OUT_F = 11008
N_CORES = 8
O_SH = OUT_F // N_CORES  # 1376
KT = IN_F // 128         # 32 k-tiles
MS = 256                 # tokens per x-slab DMA

_PROG = {}


def _build(M, O, kt):
    import concourse.mybir as mybir
    import concourse.tile as tile
    from concourse import bacc

    f16, f32, u8 = mybir.dt.float16, mybir.dt.float32, mybir.dt.uint8
    nc = bacc.Bacc("TRN2", target_bir_lowering=False, debug=False,
                   num_devices=N_CORES)
    K = kt * 128
    xT = nc.dram_tensor("xT", (K, M), f16, kind="ExternalInput")
    wq = nc.dram_tensor("wq", (K, O), u8, kind="ExternalInput")
    s2 = nc.dram_tensor("s2", (kt * 4, O), f16, kind="ExternalInput")
    bb = nc.dram_tensor("bb", (128, O), f32, kind="ExternalInput")
    y = nc.dram_tensor("y", (M, O), f32, kind="ExternalOutput")

    chunks = [(o0, min(512, O - o0)) for o0 in range(0, O, 512)]
    ms_cnt = M // MS

    with tile.TileContext(nc) as tc:
        with (
            tc.tile_pool(name="wres", bufs=1) as wres,
            tc.tile_pool(name="consts", bufs=1) as consts,
            tc.tile_pool(name="qp", bufs=2) as qp,
            tc.tile_pool(name="sp", bufs=2) as sp,
            tc.tile_pool(name="xp", bufs=2) as xp,
            tc.tile_pool(name="op", bufs=3) as op,
            tc.tile_pool(name="pp", bufs=4, space="PSUM") as pp,
        ):
            w_all = wres.tile([128, kt, O], f16)
            bias_sb = consts.tile([128, O], f32)
            nc.sync.dma_start(out=bias_sb, in_=bb[:, :])

            wq_r = wq.rearrange("(t p) o -> t p o", p=128)
            for t in range(kt):
                qt = qp.tile([128, O], u8)
                nc.sync.dma_start(out=qt, in_=wq_r[t])
                st = sp.tile([128, O], f16)
                for pg in range(4):
                    nc.gpsimd.dma_start(
                        out=st[pg * 32:(pg + 1) * 32, :],
                        in_=s2[4 * t + pg:4 * t + pg + 1, :].to_broadcast((32, O)),
                    )
                # w = (q - 7.5) * (2*norm/15)
                qf = sp.tile([128, O], f16, tag="qf")
                nc.scalar.activation(
                    qf, qt, mybir.ActivationFunctionType.Copy, bias=-7.5)
                nc.vector.tensor_mul(w_all[:, t, :], qf, st)

            xT_r = xT.rearrange("(t p) m -> p t m", p=128)
            for ms in range(ms_cnt):
                xs = xp.tile([128, kt, MS], f16)
                nc.sync.dma_start(out=xs, in_=xT_r[:, :, ms * MS:(ms + 1) * MS])
                for mt in range(MS // 128):
                    m0 = ms * MS + mt * 128
                    ob = op.tile([128, O], f32)
                    for (o0, on) in chunks:
                        ps = pp.tile([128, 512], f32, tag="ps")
                        for t in range(kt):
                            nc.tensor.matmul(
                                ps[:, :on],
                                xs[:, t, mt * 128:(mt + 1) * 128],
                                w_all[:, t, o0:o0 + on],
                                start=(t == 0), stop=(t == kt - 1),
                            )
                        nc.vector.tensor_add(
                            ob[:, o0:o0 + on], ps[:, :on], bias_sb[:, o0:o0 + on])
                    nc.sync.dma_start(out=y[m0:m0 + 128, :], in_=ob)
    nc.compile()
    return nc


def _get_prog(M=None, O=None, kt=None):
    key = (M or 8192, O or O_SH, kt or KT)
    if key not in _PROG:
        _PROG[key] = _build(*key)
    return _PROG[key]


def _in_maps(x, weight_q4, weight_norm, bias, n_cores=N_CORES):
    x = np.asarray(x)
    M = x.size // IN_F
    X = np.asarray(x, np.float32).reshape(M, IN_F)
    xT = np.ascontiguousarray(X.T).astype(np.float16)  # (4096, M)

    q = np.asarray(weight_q4).astype(np.uint8)          # (O, 128, 16)
    low = q & 15
    high = q >> 4
    w8 = np.stack((low, high), axis=-1).reshape(OUT_F, IN_F)
    wqT = np.ascontiguousarray(w8.T)                    # (4096, 11008) u8

    nf = np.asarray(weight_norm, np.float32)[:, :, 0]   # (344, 128)
    s2o = np.repeat(nf * (2.0 / 15.0), 32, axis=0)      # (11008, 128)
    s2T = np.ascontiguousarray(s2o.T).astype(np.float16)  # (128, 11008)

    bias = np.asarray(bias, np.float32)
    o_sh = OUT_F // n_cores
    maps = []
    for c in range(n_cores):
        sl = slice(c * o_sh, (c + 1) * o_sh)
        maps.append({
            "xT": xT,
            "wq": np.ascontiguousarray(wqT[:, sl]),
            "s2": np.ascontiguousarray(s2T[:, sl]),
            "bb": np.ascontiguousarray(
                np.broadcast_to(bias[sl], (128, o_sh))),
        })
    return maps


def kernel(x, weight_q4, weight_norm, bias):
    from concourse.bass_utils import run_bass_kernel_spmd
    x = np.asarray(x)
    maps = _in_maps(x, weight_q4, weight_norm, bias)
    nc = _get_prog(M=x.size // IN_F)
    res = run_bass_kernel_spmd(nc, maps, core_ids=list(range(N_CORES)))
    out = np.concatenate([r["y"] for r in res.results], axis=1)
    return out.reshape(x.shape[0], x.shape[1], OUT_F)


# revision 10
# speedup vs baseline: 1.0467x; 1.0467x over previous
"""Bass/Trainium2 kernel for LinearRowShared4Bit.

y[b,s,o] = sum_i x[b,s,i] * W[o,i] + bias[o]
W[o,i]   = (2*q[o,i]/15 - 1) * norm[o//32, i//32]   (q = 4-bit nibbles)

Sharding: out_features (11008) split 1376-per-core across 8 cores; x replicated.

Host does layout only (transpose / nibble->byte unpack / norm expansion);
all dequant arithmetic ((q-7.5)*(2*norm/15)) and the matmul run on device.

Device per core:
  - dequantize W into a resident SBUF tile [128, 32, 1376] fp16 (11.3 MB)
  - for each 128-token tile: accumulate 3 o-chunks x 32 k-tile matmuls in PSUM
    (lhsT = xT k-tile [128,128], rhs = W k-tile chunk [128,<=512]),
    add bias on DVE, DMA out fp32 rows.
"""

import numpy as np

IN_F = 4096
OUT_F = 11008
N_CORES = 8
O_SH = OUT_F // N_CORES  # 1376
KT = IN_F // 128         # 32 k-tiles
MS = 256                 # tokens per x-slab DMA

_PROG = {}


def _build(M, O, kt):
    import concourse.mybir as mybir
    import concourse.tile as tile
    from concourse import bacc

    f16, f32, u8 = mybir.dt.float16, mybir.dt.float32, mybir.dt.uint8
    nc = bacc.Bacc("TRN2", target_bir_lowering=False, debug=False,
                   num_devices=N_CORES)
    K = kt * 128
    xT = nc.dram_tensor("xT", (K, M), f16, kind="ExternalInput")
    wq = nc.dram_tensor("wq", (K, O), u8, kind="ExternalInput")
    s2 = nc.dram_tensor("s2", (kt * 4, O), f16, kind="ExternalInput")
    ee = nc.dram_tensor("ee", (128, kt * 128), f16, kind="ExternalInput")
    bb = nc.dram_tensor("bb", (128, O), f32, kind="ExternalInput")
    y = nc.dram_tensor("y", (M, O), f32, kind="ExternalOutput")

    chunks = [(o0, min(512, O - o0)) for o0 in range(0, O, 512)]
    ms_cnt = M // MS

    with tile.TileContext(nc) as tc:
        with (
            tc.tile_pool(name="wres", bufs=1) as wres,
            tc.tile_pool(name="consts", bufs=1) as consts,
            tc.tile_pool(name="qp", bufs=8) as qp,
            tc.tile_pool(name="xp", bufs=2) as xp,
            tc.tile_pool(name="op", bufs=3) as op,
            tc.tile_pool(name="pp", bufs=4, space="PSUM") as pp,
            tc.tile_pool(name="spp", bufs=2, space="PSUM") as spp,
        ):
            w_all = wres.tile([128, kt, O], f16)
            bias_sb = consts.tile([128, O], f32)
            nc.sync.dma_start(out=bias_sb, in_=bb[:, :])
            s2_sb = consts.tile([kt * 4, O], f16)
            nc.sync.dma_start(out=s2_sb, in_=s2[:, :])
            e_all = consts.tile([128, kt, 128], f16)
            nc.sync.dma_start(out=e_all, in_=ee.rearrange("p (t q) -> p t q", q=128))

            wq_r = wq.rearrange("(t p) o -> t p o", p=128)
            qts = []
            for t in range(kt):
                qt = qp.tile([128, O], u8)
                nc.sync.dma_start(out=qt, in_=wq_r[t])
                qts.append(qt)
            for t in range(kt):
                # S[p, o] = s2[4t + p//32, o] via one-hot matmul, then
                # w = (q - 7.5) * S
                for (o0, on) in chunks:
                    sps = spp.tile([128, 512], f32, tag="sps")
                    nc.tensor.matmul(
                        sps[:, :on], e_all[:, t, :], s2_sb[:, o0:o0 + on],
                        start=True, stop=True)
                    nc.vector.scalar_tensor_tensor(
                        w_all[:, t, o0:o0 + on], qts[t][:, o0:o0 + on],
                        7.5, sps[:, :on],
                        op0=mybir.AluOpType.subtract, op1=mybir.AluOpType.mult)

            xT_r = xT.rearrange("(t p) m -> p t m", p=128)
            for ms in range(ms_cnt):
                xs = xp.tile([128, kt, MS], f16)
                nc.sync.dma_start(out=xs, in_=xT_r[:, :, ms * MS:(ms + 1) * MS])
                for mt in range(MS // 128):
                    m0 = ms * MS + mt * 128
                    ob = op.tile([128, O], f32)
                    for (o0, on) in chunks:
                        ps = pp.tile([128, 512], f32, tag="ps")
                        for t in range(kt):
                            nc.tensor.matmul(
                                ps[:, :on],
                                xs[:, t, mt * 128:(mt + 1) * 128],
                                w_all[:, t, o0:o0 + on],
                                start=(t == 0), stop=(t == kt - 1),
                            )
                        nc.vector.tensor_add(
                            ob[:, o0:o0 + on], ps[:, :on], bias_sb[:, o0:o0 + on])
                    nc.sync.dma_start(out=y[m0:m0 + 128, :], in_=ob)
    nc.compile()
    return nc


def _get_prog(M=None, O=None, kt=None):
    key = (M or 8192, O or O_SH, kt or KT)
    if key not in _PROG:
        _PROG[key] = _build(*key)
    return _PROG[key]


def _in_maps(x, weight_q4, weight_norm, bias, n_cores=N_CORES):
    x = np.asarray(x)
    M = x.size // IN_F
    X = np.asarray(x, np.float32).reshape(M, IN_F)
    xT = np.ascontiguousarray(X.T).astype(np.float16)  # (4096, M)

    q = np.asarray(weight_q4).astype(np.uint8)          # (O, 128, 16)
    low = q & 15
    high = q >> 4
    w8 = np.stack((low, high), axis=-1).reshape(OUT_F, IN_F)
    wqT = np.ascontiguousarray(w8.T)                    # (4096, 11008) u8

    nf = np.asarray(weight_norm, np.float32)[:, :, 0]   # (344, 128)
    s2o = np.repeat(nf * (2.0 / 15.0), 32, axis=0)      # (11008, 128)
    s2T = np.ascontiguousarray(s2o.T).astype(np.float16)  # (128, 11008)

    bias = np.asarray(bias, np.float32)

    # one-hot matrices for on-device scale expansion:
    # E_t[r, p] = 1 iff r == 4t + p//32  ->  (E_t.T @ s2)[p, o] = s2[4t+p//32, o]
    kt = IN_F // 128
    e_host = np.zeros((128, kt, 128), np.float16)
    p_idx = np.arange(128)
    for t in range(kt):
        e_host[4 * t + p_idx // 32, t, p_idx] = 1.0
    e_host = e_host.reshape(128, kt * 128)

    o_sh = OUT_F // n_cores
    maps = []
    for c in range(n_cores):
        sl = slice(c * o_sh, (c + 1) * o_sh)
        maps.append({
            "xT": xT,
            "wq": np.ascontiguousarray(wqT[:, sl]),
            "s2": np.ascontiguousarray(s2T[:, sl]),
            "ee": e_host,
            "bb": np.ascontiguousarray(
                np.broadcast_to(bias[sl], (128, o_sh))),
        })
    return maps


def kernel(x, weight_q4, weight_norm, bias):
    from concourse.bass_utils import run_bass_kernel_spmd
    x = np.asarray(x)
    maps = _in_maps(x, weight_q4, weight_norm, bias)
    nc = _get_prog(M=x.size // IN_F)
    res = run_bass_kernel_spmd(nc, maps, core_ids=list(range(N_CORES)))
    out = np.concatenate([r["y"] for r in res.results], axis=1)
    return out.reshape(x.shape[0], x.shape[1], OUT_F)


# revision 12
# speedup vs baseline: 1.0493x; 1.0025x over previous
"""Bass/Trainium2 kernel for LinearRowShared4Bit.

y[b,s,o] = sum_i x[b,s,i] * W[o,i] + bias[o]
W[o,i]   = (2*q[o,i]/15 - 1) * norm[o//32, i//32]   (q = 4-bit nibbles)

Sharding: out_features (11008) split 1376-per-core across 8 cores; x replicated.

Host does layout only (transpose / nibble->byte unpack / norm expansion);
all dequant arithmetic ((q-7.5)*(2*norm/15)) and the matmul run on device.

Device per core:
  - dequantize W into a resident SBUF tile [128, 32, 1376] fp16 (11.3 MB)
  - for each 128-token tile: accumulate 3 o-chunks x 32 k-tile matmuls in PSUM
    (lhsT = xT k-tile [128,128], rhs = W k-tile chunk [128,<=512]),
    add bias on DVE, DMA out fp32 rows.
"""

import numpy as np

IN_F = 4096
OUT_F = 11008
N_CORES = 8
O_SH = OUT_F // N_CORES  # 1376
KT = IN_F // 128         # 32 k-tiles
MS = 256                 # tokens per x-slab DMA

_PROG = {}


def _build(M, O, kt):
    import concourse.mybir as mybir
    import concourse.tile as tile
    from concourse import bacc

    f16, f32, u8 = mybir.dt.float16, mybir.dt.float32, mybir.dt.uint8
    nc = bacc.Bacc("TRN2", target_bir_lowering=False, debug=False,
                   num_devices=N_CORES)
    K = kt * 128
    xT = nc.dram_tensor("xT", (K, M), f16, kind="ExternalInput")
    wq = nc.dram_tensor("wq", (K, O), u8, kind="ExternalInput")
    s2 = nc.dram_tensor("s2", (kt * 4, O), f16, kind="ExternalInput")
    ee = nc.dram_tensor("ee", (128, kt * 128), f16, kind="ExternalInput")
    bb = nc.dram_tensor("bb", (128, O), f32, kind="ExternalInput")
    y = nc.dram_tensor("y", (M, O), f32, kind="ExternalOutput")

    chunks = [(o0, min(512, O - o0)) for o0 in range(0, O, 512)]
    ms_cnt = M // MS

    with tile.TileContext(nc) as tc:
        with (
            tc.tile_pool(name="wres", bufs=1) as wres,
            tc.tile_pool(name="consts", bufs=1) as consts,
            tc.tile_pool(name="qp", bufs=8) as qp,
            tc.tile_pool(name="xp", bufs=2) as xp,
            tc.tile_pool(name="op", bufs=3) as op,
            tc.tile_pool(name="pp", bufs=4, space="PSUM") as pp,
            tc.tile_pool(name="spp", bufs=3, space="PSUM") as spp,
        ):
            w_all = wres.tile([128, kt, O], f16)
            bias_sb = consts.tile([128, O], f32)
            nc.sync.dma_start(out=bias_sb, in_=bb[:, :])
            s2_sb = consts.tile([kt * 4, O], f16)
            nc.sync.dma_start(out=s2_sb, in_=s2[:, :])
            e_all = consts.tile([128, kt, 128], f16)
            nc.sync.dma_start(out=e_all, in_=ee.rearrange("p (t q) -> p t q", q=128))

            wq_r = wq.rearrange("(t p) o -> t p o", p=128)
            qts = []
            for t in range(kt):
                qt = qp.tile([128, O], u8)
                nc.sync.dma_start(out=qt, in_=wq_r[t])
                qts.append(qt)
            for t in range(kt):
                # S[p, o] = s2[4t + p//32, o] via one-hot matmul, then
                # w = (q - 7.5) * S
                for (o0, on) in chunks:
                    sps = spp.tile([128, 512], f32, tag="sps")
                    nc.tensor.matmul(
                        sps[:, :on], e_all[:, t, :], s2_sb[:, o0:o0 + on],
                        start=True, stop=True)
                    nc.vector.scalar_tensor_tensor(
                        w_all[:, t, o0:o0 + on], qts[t][:, o0:o0 + on],
                        7.5, sps[:, :on],
                        op0=mybir.AluOpType.subtract, op1=mybir.AluOpType.mult)

            xT_r = xT.rearrange("(t p) m -> p t m", p=128)
            for ms in range(ms_cnt):
                xs = xp.tile([128, kt, MS], f16)
                nc.sync.dma_start(out=xs, in_=xT_r[:, :, ms * MS:(ms + 1) * MS])
                for mt in range(MS // 128):
                    m0 = ms * MS + mt * 128
                    ob = op.tile([128, O], f32)
                    for (o0, on) in chunks:
                        ps = pp.tile([128, 512], f32, tag="ps")
                        for t in range(kt):
                            nc.tensor.matmul(
                                ps[:, :on],
                                xs[:, t, mt * 128:(mt + 1) * 128],
                                w_all[:, t, o0:o0 + on],
                                start=(t == 0), stop=(t == kt - 1),
                            )
                        nc.vector.tensor_add(
                            ob[:, o0:o0 + on], ps[:, :on], bias_sb[:, o0:o0 + on])
                        nc.sync.dma_start(
                            out=y[m0:m0 + 128, o0:o0 + on],
                            in_=ob[:, o0:o0 + on])
    nc.compile()
    return nc


def _get_prog(M=None, O=None, kt=None):
    key = (M or 8192, O or O_SH, kt or KT)
    if key not in _PROG:
        _PROG[key] = _build(*key)
    return _PROG[key]


def _in_maps(x, weight_q4, weight_norm, bias, n_cores=N_CORES):
    x = np.asarray(x)
    M = x.size // IN_F
    X = np.asarray(x, np.float32).reshape(M, IN_F)
    xT = np.ascontiguousarray(X.T).astype(np.float16)  # (4096, M)

    q = np.asarray(weight_q4).astype(np.uint8)          # (O, 128, 16)
    low = q & 15
    high = q >> 4
    w8 = np.stack((low, high), axis=-1).reshape(OUT_F, IN_F)
    wqT = np.ascontiguousarray(w8.T)                    # (4096, 11008) u8

    nf = np.asarray(weight_norm, np.float32)[:, :, 0]   # (344, 128)
    s2o = np.repeat(nf * (2.0 / 15.0), 32, axis=0)      # (11008, 128)
    s2T = np.ascontiguousarray(s2o.T).astype(np.float16)  # (128, 11008)

    bias = np.asarray(bias, np.float32)

    # one-hot matrices for on-device scale expansion:
    # E_t[r, p] = 1 iff r == 4t + p//32  ->  (E_t.T @ s2)[p, o] = s2[4t+p//32, o]
    kt = IN_F // 128
    e_host = np.zeros((128, kt, 128), np.float16)
    p_idx = np.arange(128)
    for t in range(kt):
        e_host[4 * t + p_idx // 32, t, p_idx] = 1.0
    e_host = e_host.reshape(128, kt * 128)

    o_sh = OUT_F // n_cores
    maps = []
    for c in range(n_cores):
        sl = slice(c * o_sh, (c + 1) * o_sh)
        maps.append({
            "xT": xT,
            "wq": np.ascontiguousarray(wqT[:, sl]),
            "s2": np.ascontiguousarray(s2T[:, sl]),
            "ee": e_host,
            "bb": np.ascontiguousarray(
                np.broadcast_to(bias[sl], (128, o_sh))),
        })
    return maps


def kernel(x, weight_q4, weight_norm, bias):
    from concourse.bass_utils import run_bass_kernel_spmd
    x = np.asarray(x)
    maps = _in_maps(x, weight_q4, weight_norm, bias)
    nc = _get_prog(M=x.size // IN_F)
    res = run_bass_kernel_spmd(nc, maps, core_ids=list(range(N_CORES)))
    out = np.concatenate([r["y"] for r in res.results], axis=1)
    return out.reshape(x.shape[0], x.shape[1], OUT_F)
